# revision 1
# baseline (speedup 1.0000x reference)
"""Trainium2 Bass kernel for ExpressionAutoDiscretization (embedding_lookup).

Reference computation, per token t (B=8, N=19264, BIN=100, D=768):
    v1 = x_t * w1 + b1                      # (100,)
    v2 = leaky_relu(v1, 0.1)
    v3 = v2 + w2 @ v2 + b2
    w  = softmax(v3)
    e  = w @ emb_table                      # (768,)
    e  = pad_emb.bf16  if pad_mask  else e
    e  = mask_emb.bf16 if masked_mask else e   (mask wins over pad)

Kernel strategy (pure data parallel, batch row b -> core b):
  * bins live on the SBUF partition axis; tokens on the free axis.
  * mm1: [1,100].T @ [1,T] -> v1 [100,T] (K=1 outer product)
  * leaky relu + b1 on DVE (3 ops), giving v2 rows 0..99 of a [101,T] tile
    whose row 100 is a host-computed per-token penalty row `pen`:
       pen = -5      for live tokens   (global softmax shift, exactness-
                                        invariant, keeps denom in the DVE
                                        reciprocal range)
       pen = -30000  for pad/mask tokens (zeroes their exp weights)
  * mm2: lhsT = [(w2.T + I); ones] (101,100)  ->  v3 [100,T] with pen folded
  * ACT Exp with per-partition bias b2 -> E rows 0..99 of a [102,T] tile;
    rows 100/101 are host indicator rows p = pad&!mask, q = mask.
  * mm3: lhsT = E[:, chunk] (102,112), rhs = emb_aug (102,769) where
       emb_aug = [[emb_table, 1], [pad_emb.bf16, 1], [mask_emb.bf16, 1]]
    -> out_psum [112, 769]; col 768 is the softmax denominator (== 1 for
    masked tokens, making them bit-exact pad/mask embeddings).
  * DVE reciprocal of col 768, per-partition multiply, DMA out.

All matmuls run in full fp32 (4-pass PE mode).
"""

import numpy as np
import ml_dtypes

B = 8
N = 19264          # tokens per core (= one batch row)
BIN = 100
D = 768
ST = 448           # supertile (free-dim token count); 43 * 448 = 19264
NST = N // ST
CH = 112           # mm3 output chunk (partition dim); 4 * 112 = 448
NCH = ST // CH
PEN_LIVE = -5.0
PEN_DEAD = -30000.0

_prog_cache = {}


def _build_program(mm3_dtype="float32", mm12_dtype="float32", div_act_every=4):
    import concourse.bacc as bacc
    import concourse.bass as bass
    import concourse.mybir as mybir
    import concourse.tile as tile

    f32 = mybir.dt.float32
    AF = mybir.ActivationFunctionType
    Alu = mybir.AluOpType

    nc = bacc.Bacc(
        "TRN2",
        target_bir_lowering=False,
        debug=False,
        enable_asserts=True,
        num_devices=B,
    )

    x_d = nc.dram_tensor("x", [1, N], f32, kind="ExternalInput")
    pen_d = nc.dram_tensor("pen", [1, N], f32, kind="ExternalInput")
    pq_d = nc.dram_tensor("pq", [2, N], f32, kind="ExternalInput")
    w1_d = nc.dram_tensor("w1", [1, BIN], f32, kind="ExternalInput")
    w2i_d = nc.dram_tensor("w2i", [BIN + 1, BIN], f32, kind="ExternalInput")
    emb_d = nc.dram_tensor("emb", [BIN + 2, D + 1], f32, kind="ExternalInput")
    b1_d = nc.dram_tensor("b1", [BIN, 1], f32, kind="ExternalInput")
    b2_d = nc.dram_tensor("b2", [BIN, 1], f32, kind="ExternalInput")
    y_d = nc.dram_tensor("y", [N, D], f32, kind="ExternalOutput")

    with tile.TileContext(nc) as tc:
        with (
            tc.tile_pool(name="consts", bufs=1) as consts,
            tc.tile_pool(name="xp", bufs=4) as xp,
            tc.tile_pool(name="v2p", bufs=3) as v2p,
            tc.tile_pool(name="ep", bufs=3) as ep,
            tc.tile_pool(name="scr", bufs=3) as scr,
            tc.tile_pool(name="outs", bufs=6) as outs,
            tc.tile_pool(name="rp", bufs=8) as rp,
            tc.tile_pool(name="v1ps", bufs=2, space="PSUM") as v1ps,
            tc.tile_pool(name="v3ps", bufs=2, space="PSUM") as v3ps,
            tc.tile_pool(name="ops", bufs=2, space="PSUM") as ops,
        ):
            w1_t = consts.tile([1, BIN], f32)
            w2i_t = consts.tile([BIN + 1, BIN], f32)
            emb_t = consts.tile([BIN + 2, D + 1], f32)
            b1_t = consts.tile([BIN, 1], f32)
            b2_t = consts.tile([BIN, 1], f32)
            nc.sync.dma_start(w1_t[:], w1_d[:])
            nc.sync.dma_start(w2i_t[:], w2i_d[:])
            nc.sync.dma_start(emb_t[:], emb_d[:])
            nc.sync.dma_start(b1_t[:], b1_d[:])
            nc.sync.dma_start(b2_t[:], b2_d[:])

            for i in range(NST):
                t0 = i * ST
                x_t = xp.tile([1, ST], f32)
                nc.sync.dma_start(x_t[:], x_d[0:1, t0:t0 + ST])

                v2_t = v2p.tile([BIN + 1, ST], f32)
                nc.sync.dma_start(v2_t[BIN:BIN + 1, :], pen_d[0:1, t0:t0 + ST])

                e_t = ep.tile([BIN + 2, ST], f32)
                nc.sync.dma_start(e_t[BIN:BIN + 2, :], pq_d[0:2, t0:t0 + ST])

                # mm1: v1 = w1 (x) x   -> [100, ST]
                v1_p = v1ps.tile([BIN, ST], f32)
                nc.tensor.matmul(v1_p[:], w1_t[:], x_t[:], start=True, stop=True)

                # leaky relu: v2 = max(v1 + b1, 0.1*(v1 + b1))
                a_t = scr.tile([BIN, ST], f32)
                nc.vector.tensor_scalar(
                    out=a_t[:], in0=v1_p[:], scalar1=b1_t[:], scalar2=None,
                    op0=Alu.add,
                )
                nc.vector.tensor_scalar(
                    out=v2_t[0:BIN, :], in0=v1_p[:], scalar1=b1_t[:],
                    scalar2=0.1, op0=Alu.add, op1=Alu.mult,
                )
                nc.vector.tensor_tensor(
                    out=v2_t[0:BIN, :], in0=v2_t[0:BIN, :], in1=a_t[:],
                    op=Alu.max,
                )

                # mm2: v3 = (w2.T + I).T @ v2 + pen -> [100, ST]
                v3_p = v3ps.tile([BIN, ST], f32)
                nc.tensor.matmul(v3_p[:], w2i_t[:], v2_t[:], start=True, stop=True)

                # E = exp(v3 + b2)
                nc.scalar.activation(e_t[0:BIN, :], v3_p[:], AF.Exp, bias=b2_t[:])

                for c in range(NCH):
                    cs = c * CH
                    o_p = ops.tile([CH, D + 1], f32)
                    nc.tensor.matmul(
                        o_p[:, 0:512], e_t[:, cs:cs + CH], emb_t[:, 0:512],
                        start=True, stop=True,
                    )
                    nc.tensor.matmul(
                        o_p[:, 512:D + 1], e_t[:, cs:cs + CH], emb_t[:, 512:D + 1],
                        start=True, stop=True,
                    )
                    r_t = rp.tile([CH, 1], f32)
                    nc.vector.reciprocal(r_t[:], o_p[:, D:D + 1])
                    o_s = outs.tile([CH, D], f32)
                    k = i * NCH + c
                    if div_act_every and k % div_act_every == div_act_every - 1:
                        nc.scalar.mul(o_s[:], o_p[:, 0:D], r_t[:])
                    else:
                        nc.vector.tensor_scalar(
                            out=o_s[:], in0=o_p[:, 0:D], scalar1=r_t[:],
                            scalar2=None, op0=Alu.mult,
                        )
                    nc.sync.dma_start(y_d[t0 + cs:t0 + cs + CH, :], o_s[:])

    nc.compile()
    return nc


def _preprocess(inputs):
    ge = np.ascontiguousarray(np.asarray(inputs["gene_expression"], dtype=np.float32))
    pad = np.asarray(inputs["pad_mask"]) != 0
    msk = np.asarray(inputs["masked_mask"]) != 0
    w1 = np.asarray(inputs["w1"], dtype=np.float32)
    b1 = np.asarray(inputs["b1"], dtype=np.float32)
    w2 = np.asarray(inputs["w2"], dtype=np.float32)
    b2 = np.asarray(inputs["b2"], dtype=np.float32)
    emb = np.asarray(inputs["emb_table"], dtype=np.float32)
    pad_e = np.asarray(inputs["pad_emb"], dtype=np.float32)
    mask_e = np.asarray(inputs["mask_emb"], dtype=np.float32)

    pad_e = pad_e.astype(ml_dtypes.bfloat16).astype(np.float32)
    mask_e = mask_e.astype(ml_dtypes.bfloat16).astype(np.float32)

    dead = pad | msk
    pen = np.where(dead, PEN_DEAD, PEN_LIVE).astype(np.float32)     # (B, N)
    q = msk.astype(np.float32)                                      # (B, N)
    p = (pad & ~msk).astype(np.float32)                             # (B, N)

    w2i = np.concatenate(
        [w2.T + np.eye(BIN, dtype=np.float32), np.ones((1, BIN), np.float32)],
        axis=0,
    ).astype(np.float32)                                            # (101, 100)

    emb_aug = np.ones((BIN + 2, D + 1), np.float32)
    emb_aug[0:BIN, 0:D] = emb
    emb_aug[BIN, 0:D] = pad_e
    emb_aug[BIN + 1, 0:D] = mask_e                                  # col D stays 1.0

    consts = {
        "w1": np.ascontiguousarray(w1[None, :]),
        "w2i": np.ascontiguousarray(w2i),
        "emb": np.ascontiguousarray(emb_aug),
        "b1": np.ascontiguousarray(b1[:, None]),
        "b2": np.ascontiguousarray(b2[:, None]),
    }
    in_maps = []
    for b in range(B):
        m = dict(consts)
        m["x"] = np.ascontiguousarray(ge[b][None, :])
        m["pen"] = np.ascontiguousarray(pen[b][None, :])
        m["pq"] = np.ascontiguousarray(np.stack([p[b], q[b]], axis=0))
        in_maps.append(m)
    return in_maps


def _run(inputs, trace=False, trace_cores=None, **kw):
    from concourse.bass_utils import run_bass_kernel_spmd

    key = "v1"
    if key not in _prog_cache:
        _prog_cache[key] = _build_program()
    nc = _prog_cache[key]
    in_maps = _preprocess(inputs)
    res = run_bass_kernel_spmd(
        nc, in_maps, core_ids=list(range(B)),
        trace=trace, trace_cores=trace_cores, **kw,
    )
    out = np.stack([res.results[b]["y"] for b in range(B)], axis=0)
    return out, res


def kernel(**inputs):
    out, _ = _run(inputs, trace=False)
    return out


# revision 4
# speedup vs baseline: 1.1846x; 1.1846x over previous
"""Trainium2 Bass kernel for ExpressionAutoDiscretization (embedding_lookup).

Reference computation, per token t (B=8, N=19264, BIN=100, D=768):
    v1 = x_t * w1 + b1                      # (100,)
    v2 = leaky_relu(v1, 0.1)
    v3 = v2 + w2 @ v2 + b2
    w  = softmax(v3)
    e  = w @ emb_table                      # (768,)
    e  = pad_emb.bf16  if pad_mask  else e
    e  = mask_emb.bf16 if masked_mask else e   (mask wins over pad)

Kernel strategy (pure data parallel, batch row b -> core b):
  * bins live on the SBUF partition axis; tokens on the free axis.
  * mm1: lhsT=[[w1],[b1]] (2,100), rhs=[[x],[1]] (2,T) -> v1+b1 [100,T]
  * leaky relu on DVE (2 ops: 0.1*z, then max)
  * mm2: lhsT = [(w2.T + I); ones] (101,100), rhs = [v2; pen] (101,T)
    pen is a host-computed per-token penalty row:
       pen = -5      live tokens (global softmax shift, keeps exp sums in
                     the DVE reciprocal range; exactness-invariant)
       pen = -30000  pad/mask tokens (zeroes their exp weights)
  * ACT Exp with per-partition bias b2 -> E rows 0..99 of a [102,T] tile;
    rows 100/101 are host indicator rows p = pad&!mask, q = mask.
  * mm3 (float32r): lhsT = E[:, chunk] (102,112), rhs = emb_aug (102,769):
       emb_aug = [[emb_table, 1], [pad_emb.bf16, 1], [mask_emb.bf16, 1]]
    -> out_psum [112, 769]; col 768 is the softmax denominator (== 1 for
    masked tokens, making them bit-exact pad/mask embeddings).
  * DVE reciprocal of col 768, per-partition multiply (split ACT/DVE),
    one fused 3D-AP output DMA per supertile.
"""

import numpy as np
import ml_dtypes

B = 8
N = 19264          # tokens per core (= one batch row)
BIN = 100
D = 768
ST = 448           # supertile (free-dim token count); 43 * 448 = 19264
NST = N // ST
CH = 112           # mm3 output chunk (partition dim); 4 * 112 = 448
NCH = ST // CH
PEN_LIVE = -5.0
PEN_DEAD = -30000.0

_prog_cache = {}


def _build_program(mm3_rdc=True, mm12_rdc=False, div_act_mod=(5, 3)):
    import concourse.bacc as bacc
    import concourse.mybir as mybir
    import concourse.tile as tile

    f32 = mybir.dt.float32
    f32r = mybir.dt.float32r
    AF = mybir.ActivationFunctionType
    Alu = mybir.AluOpType

    nc = bacc.Bacc(
        "TRN2",
        target_bir_lowering=False,
        debug=False,
        enable_asserts=True,
        num_devices=B,
    )

    e_dt = f32r if mm3_rdc else f32
    x_d = nc.dram_tensor("x", [2, N], f32, kind="ExternalInput")     # [x; ones]
    pen_d = nc.dram_tensor("pen", [1, N], f32, kind="ExternalInput")
    pq_d = nc.dram_tensor("pq", [2, N], e_dt, kind="ExternalInput")  # [p; q]
    EW = D + 2  # emb_aug width: 768 data + denom col + even-N pad
    w1_d = nc.dram_tensor("w1", [2, BIN], f32, kind="ExternalInput")  # [w1; b1]
    w2i_d = nc.dram_tensor("w2i", [BIN + 1, BIN], f32, kind="ExternalInput")
    emb_d = nc.dram_tensor("emb", [BIN + 2, EW], e_dt, kind="ExternalInput")
    b2_d = nc.dram_tensor("b2", [BIN, 1], f32, kind="ExternalInput")
    y_d = nc.dram_tensor("y", [N, D], f32, kind="ExternalOutput")

    with tile.TileContext(nc) as tc:
        with (
            tc.tile_pool(name="consts", bufs=1) as consts,
            tc.tile_pool(name="xp", bufs=4) as xp,
            tc.tile_pool(name="v2p", bufs=3) as v2p,
            tc.tile_pool(name="ep", bufs=3) as ep,
            tc.tile_pool(name="scr", bufs=3) as scr,
            tc.tile_pool(name="outs", bufs=3) as outs,
            tc.tile_pool(name="rp", bufs=8) as rp,
            tc.tile_pool(name="v1ps", bufs=2, space="PSUM") as v1ps,
            tc.tile_pool(name="v3ps", bufs=2, space="PSUM") as v3ps,
            tc.tile_pool(name="ops", bufs=2, space="PSUM") as ops,
        ):
            w1_t = consts.tile([2, BIN], f32)
            w2i_t = consts.tile([BIN + 1, BIN], f32)
            emb_t = consts.tile([BIN + 2, EW], e_dt)
            b2_t = consts.tile([BIN, 1], f32)
            nc.sync.dma_start(w1_t[:], w1_d[:])
            nc.sync.dma_start(w2i_t[:], w2i_d[:])
            nc.sync.dma_start(emb_t[:], emb_d[:])
            nc.sync.dma_start(b2_t[:], b2_d[:])

            for i in range(NST):
                t0 = i * ST
                x_t = xp.tile([2, ST], f32)
                nc.gpsimd.dma_start(x_t[:], x_d[0:2, t0:t0 + ST])

                v2_t = v2p.tile([BIN + 1, ST], f32)
                nc.gpsimd.dma_start(v2_t[BIN:BIN + 1, :], pen_d[0:1, t0:t0 + ST])

                e_t = ep.tile([BIN + 2, ST], e_dt)
                nc.gpsimd.dma_start(e_t[BIN:BIN + 2, :], pq_d[0:2, t0:t0 + ST])

                # mm1: v1 = w1 (x) x + b1  -> [100, ST]
                v1_p = v1ps.tile([BIN, ST], f32)
                nc.tensor.matmul(
                    v1_p[:], w1_t[:], x_t[:], start=True, stop=True,
                )

                # leaky relu: v2 = max(z, 0.1*z), z = v1 + b1
                a_t = scr.tile([BIN, ST], f32)
                nc.vector.tensor_scalar(
                    out=a_t[:], in0=v1_p[:], scalar1=0.1, scalar2=None,
                    op0=Alu.mult,
                )
                nc.vector.tensor_tensor(
                    out=v2_t[0:BIN, :], in0=v1_p[:], in1=a_t[:], op=Alu.max,
                )

                # mm2: v3 = (w2.T + I).T @ v2 + pen -> [100, ST]
                v3_p = v3ps.tile([BIN, ST], f32)
                nc.tensor.matmul(
                    v3_p[:], w2i_t[:], v2_t[:], start=True, stop=True,
                )

                # E = exp(v3 + b2)
                nc.scalar.activation(e_t[0:BIN, :], v3_p[:], AF.Exp, bias=b2_t[:])

                o_s = outs.tile([CH, NCH * D], f32)
                for c in range(NCH):
                    cs = c * CH
                    o_p = ops.tile([CH, EW], f32)
                    nc.tensor.matmul(
                        o_p[:, 0:512], e_t[:, cs:cs + CH],
                        emb_t[:, 0:512], start=True, stop=True,
                    )
                    nc.tensor.matmul(
                        o_p[:, 512:EW], e_t[:, cs:cs + CH],
                        emb_t[:, 512:EW], start=True, stop=True,
                    )
                    r_t = rp.tile([CH, 1], f32)
                    nc.vector.reciprocal(r_t[:], o_p[:, D:D + 1])
                    k = i * NCH + c
                    if k % div_act_mod[0] < div_act_mod[1]:
                        nc.scalar.mul(o_s[:, c * D:(c + 1) * D], o_p[:, 0:D], r_t[:])
                    else:
                        nc.vector.tensor_scalar(
                            out=o_s[:, c * D:(c + 1) * D], in0=o_p[:, 0:D],
                            scalar1=r_t[:], scalar2=None, op0=Alu.mult,
                        )
                dst = y_d[t0:t0 + ST, 0:D].rearrange("(c p) d -> p c d", p=CH)
                src = o_s[:].rearrange("p (c d) -> p c d", d=D)
                nc.sync.dma_start(dst, src)

    nc.compile()
    return nc


def _preprocess(inputs):
    ge = np.ascontiguousarray(np.asarray(inputs["gene_expression"], dtype=np.float32))
    pad = np.asarray(inputs["pad_mask"]) != 0
    msk = np.asarray(inputs["masked_mask"]) != 0
    w1 = np.asarray(inputs["w1"], dtype=np.float32)
    b1 = np.asarray(inputs["b1"], dtype=np.float32)
    w2 = np.asarray(inputs["w2"], dtype=np.float32)
    b2 = np.asarray(inputs["b2"], dtype=np.float32)
    emb = np.asarray(inputs["emb_table"], dtype=np.float32)
    pad_e = np.asarray(inputs["pad_emb"], dtype=np.float32)
    mask_e = np.asarray(inputs["mask_emb"], dtype=np.float32)

    pad_e = pad_e.astype(ml_dtypes.bfloat16).astype(np.float32)
    mask_e = mask_e.astype(ml_dtypes.bfloat16).astype(np.float32)

    dead = pad | msk
    pen = np.where(dead, PEN_DEAD, PEN_LIVE).astype(np.float32)     # (B, N)
    q = msk.astype(np.float32)                                      # (B, N)
    p = (pad & ~msk).astype(np.float32)                             # (B, N)

    w2i = np.concatenate(
        [w2.T + np.eye(BIN, dtype=np.float32), np.ones((1, BIN), np.float32)],
        axis=0,
    ).astype(np.float32)                                            # (101, 100)

    emb_aug = np.zeros((BIN + 2, D + 2), np.float32)
    emb_aug[:, D] = 1.0                                             # denominator col
    emb_aug[0:BIN, 0:D] = emb
    emb_aug[BIN, 0:D] = pad_e
    emb_aug[BIN + 1, 0:D] = mask_e                                  # col D+1 stays 0 (even-N pad)

    consts = {
        "w1": np.ascontiguousarray(np.stack([w1, b1], axis=0)),
        "w2i": np.ascontiguousarray(w2i),
        "emb": np.ascontiguousarray(emb_aug),
        "b2": np.ascontiguousarray(b2[:, None]),
    }
    ones = np.ones(N, np.float32)
    in_maps = []
    for b in range(B):
        m = dict(consts)
        m["x"] = np.ascontiguousarray(np.stack([ge[b], ones], axis=0))
        m["pen"] = np.ascontiguousarray(pen[b][None, :])
        m["pq"] = np.ascontiguousarray(np.stack([p[b], q[b]], axis=0))
        in_maps.append(m)
    return in_maps


def _run(inputs, trace=False, trace_cores=None, **kw):
    from concourse.bass_utils import run_bass_kernel_spmd

    key = "v2"
    if key not in _prog_cache:
        _prog_cache[key] = _build_program()
    nc = _prog_cache[key]
    in_maps = _preprocess(inputs)
    res = run_bass_kernel_spmd(
        nc, in_maps, core_ids=list(range(B)),
        trace=trace, trace_cores=trace_cores, **kw,
    )
    out = np.stack([res.results[b]["y"] for b in range(B)], axis=0)
    return out, res


def kernel(**inputs):
    out, _ = _run(inputs, trace=False)
    return out


# revision 5
# speedup vs baseline: 1.4362x; 1.2124x over previous
"""Trainium2 Bass kernel for ExpressionAutoDiscretization (embedding_lookup).

Reference computation, per token t (B=8, N=19264, BIN=100, D=768):
    v1 = x_t * w1 + b1                      # (100,)
    v2 = leaky_relu(v1, 0.1)
    v3 = v2 + w2 @ v2 + b2
    w  = softmax(v3)
    e  = w @ emb_table                      # (768,)
    e  = pad_emb.bf16  if pad_mask  else e
    e  = mask_emb.bf16 if masked_mask else e   (mask wins over pad)

Kernel strategy (pure data parallel, batch row b -> core b), bins on the
SBUF partition axis, tokens on the free axis:
  * mm1 (bf16, K=9): exact 3-way bf16 splits of x, w1, b1 (hi/mid/lo) give
    v1+b1 in fp32 PSUM to ~2^-24 — one 1-cyc/row matmul instead of fp32.
  * leaky relu z=v1+b1: a=0.1z (DVE), scr=max(z,a) fp32 (DVE),
    v2h=bf16(scr) (ACT copy), v2l=bf16(scr-v2h) (DVE).
  * mm2 = three accumulating bf16 matmuls: Whi@[v2h;pen] + Whi@v2l + Wlo@v2h
    where W = (w2.T + I) hi/lo-split on host; ones row folds the per-token
    penalty row pen into v3:
       pen = -5      live tokens (softmax shift, exactness-invariant)
       pen = -30000  pad/mask tokens (zeroes their exp weights)
  * ACT Exp(v3 + b2) -> float32r E rows 0..99 of a [102,T] tile; rows
    100/101 are host indicator rows p = pad&!mask, q = mask.
  * mm3 (float32r): lhsT = E[:, chunk] (102,<=128), rhs = emb_aug (102,770):
       emb_aug = [[emb_table, 1, 0], [pad_emb.bf16, 1, 0], [mask_emb.bf16, 1, 0]]
    -> out_psum [chunk, 770]; col 768 is the softmax denominator (== 1 for
    masked tokens, making them bit-exact pad/mask embeddings).
  * DVE reciprocal of col 768, per-partition multiply (alternating ACT/DVE),
    fused 3D-AP output DMA per supertile.
"""

import numpy as np
import ml_dtypes

BF16 = ml_dtypes.bfloat16
B = 8
N = 19264          # tokens per core (= one batch row)
BIN = 100
D = 768
EW = D + 2         # emb_aug width: 768 data + denom col + even-N pad
ST = 512           # main supertile; 37 * 512 + 320 tail
CH = 128           # mm3 output chunk (partition dim)
PEN_LIVE = -5.0
PEN_DEAD = -30000.0

_prog_cache = {}


def _blocks():
    out = []
    t0 = 0
    while t0 + ST <= N:
        out.append((t0, ST, [CH] * (ST // CH)))
        t0 += ST
    rem = N - t0
    if rem:
        chunks = [CH] * (rem // CH)
        if rem % CH:
            chunks.append(rem % CH)
        out.append((t0, rem, chunks))
    return out


def _build_program():
    import concourse.bacc as bacc
    import concourse.mybir as mybir
    import concourse.tile as tile

    f32 = mybir.dt.float32
    f32r = mybir.dt.float32r
    bf16 = mybir.dt.bfloat16
    AF = mybir.ActivationFunctionType
    Alu = mybir.AluOpType

    nc = bacc.Bacc(
        "TRN2",
        target_bir_lowering=False,
        debug=False,
        enable_asserts=True,
        num_devices=B,
    )

    xb_d = nc.dram_tensor("xb", [9, N], bf16, kind="ExternalInput")
    pen_d = nc.dram_tensor("pen", [1, N], bf16, kind="ExternalInput")
    pq_d = nc.dram_tensor("pq", [2, N], f32r, kind="ExternalInput")
    w1b_d = nc.dram_tensor("w1b", [9, BIN], bf16, kind="ExternalInput")
    whi_d = nc.dram_tensor("whi", [BIN + 1, BIN], bf16, kind="ExternalInput")
    wlo_d = nc.dram_tensor("wlo", [BIN, BIN], bf16, kind="ExternalInput")
    emb_d = nc.dram_tensor("emb", [BIN + 2, EW], f32r, kind="ExternalInput")
    b2_d = nc.dram_tensor("b2", [BIN, 1], f32, kind="ExternalInput")
    y_d = nc.dram_tensor("y", [N, D], f32, kind="ExternalOutput")

    with tile.TileContext(nc) as tc:
        with (
            tc.tile_pool(name="consts", bufs=1) as consts,
            tc.tile_pool(name="xp", bufs=4) as xp,
            tc.tile_pool(name="v2hp", bufs=3) as v2hp,
            tc.tile_pool(name="v2lp", bufs=3) as v2lp,
            tc.tile_pool(name="ep", bufs=3) as ep,
            tc.tile_pool(name="ap", bufs=3) as ap_,
            tc.tile_pool(name="scrp", bufs=3) as scrp,
            tc.tile_pool(name="outs", bufs=3) as outs,
            tc.tile_pool(name="rp", bufs=8) as rp,
            tc.tile_pool(name="v1ps", bufs=2, space="PSUM") as v1ps,
            tc.tile_pool(name="v3ps", bufs=2, space="PSUM") as v3ps,
            tc.tile_pool(name="ops", bufs=2, space="PSUM") as ops,
        ):
            w1b_t = consts.tile([9, BIN], bf16)
            whi_t = consts.tile([BIN + 1, BIN], bf16)
            wlo_t = consts.tile([BIN, BIN], bf16)
            emb_t = consts.tile([BIN + 2, EW], f32r)
            b2_t = consts.tile([BIN, 1], f32)
            nc.sync.dma_start(w1b_t[:], w1b_d[:])
            nc.sync.dma_start(whi_t[:], whi_d[:])
            nc.sync.dma_start(wlo_t[:], wlo_d[:])
            nc.sync.dma_start(emb_t[:], emb_d[:])
            nc.sync.dma_start(b2_t[:], b2_d[:])

            kdiv = 0
            for t0, st, chunks in _blocks():
                xb_t = xp.tile([9, ST], bf16)
                nc.gpsimd.dma_start(xb_t[:, 0:st], xb_d[0:9, t0:t0 + st])

                v2h_t = v2hp.tile([BIN + 1, ST], bf16)
                nc.gpsimd.dma_start(v2h_t[BIN:BIN + 1, 0:st], pen_d[0:1, t0:t0 + st])

                e_t = ep.tile([BIN + 2, ST], f32r)
                nc.gpsimd.dma_start(e_t[BIN:BIN + 2, 0:st], pq_d[0:2, t0:t0 + st])

                # mm1: z = v1 + b1 (exact via 3-way bf16 splits) -> [100, st]
                v1_p = v1ps.tile([BIN, ST], f32)
                nc.tensor.matmul(
                    v1_p[:, 0:st], w1b_t[:], xb_t[:, 0:st], start=True, stop=True,
                )

                # leaky relu + bf16 hi/lo split of v2
                a_t = ap_.tile([BIN, ST], f32)
                nc.vector.tensor_scalar(
                    out=a_t[:, 0:st], in0=v1_p[:, 0:st], scalar1=0.1, scalar2=None,
                    op0=Alu.mult,
                )
                scr_t = scrp.tile([BIN, ST], f32)
                nc.vector.tensor_tensor(
                    out=scr_t[:, 0:st], in0=v1_p[:, 0:st], in1=a_t[:, 0:st],
                    op=Alu.max,
                )
                nc.scalar.copy(v2h_t[0:BIN, 0:st], scr_t[:, 0:st])
                v2l_t = v2lp.tile([BIN, ST], bf16)
                nc.vector.tensor_tensor(
                    out=v2l_t[:, 0:st], in0=scr_t[:, 0:st], in1=v2h_t[0:BIN, 0:st],
                    op=Alu.subtract,
                )

                # mm2: v3 = Whi@[v2h;pen] + Whi@v2l + Wlo@v2h  -> [100, st]
                v3_p = v3ps.tile([BIN, ST], f32)
                nc.tensor.matmul(
                    v3_p[:, 0:st], whi_t[:], v2h_t[:, 0:st], start=True, stop=False,
                )
                nc.tensor.matmul(
                    v3_p[:, 0:st], whi_t[0:BIN, :], v2l_t[:, 0:st],
                    start=False, stop=False,
                )
                nc.tensor.matmul(
                    v3_p[:, 0:st], wlo_t[:], v2h_t[0:BIN, 0:st],
                    start=False, stop=True,
                )

                # E = exp(v3 + b2) -> float32r
                nc.scalar.activation(
                    e_t[0:BIN, 0:st], v3_p[:, 0:st], AF.Exp, bias=b2_t[:],
                )

                # mm3 + softmax divide per chunk
                nfull = sum(1 for c in chunks if c == CH)
                o_s = outs.tile([CH, len(chunks) * D], f32)
                for c, csz in enumerate(chunks):
                    cs = c * CH
                    o_p = ops.tile([CH, EW], f32)
                    nc.tensor.matmul(
                        o_p[0:csz, 0:512], e_t[:, cs:cs + csz],
                        emb_t[:, 0:512], start=True, stop=True,
                    )
                    nc.tensor.matmul(
                        o_p[0:csz, 512:EW], e_t[:, cs:cs + csz],
                        emb_t[:, 512:EW], start=True, stop=True,
                    )
                    r_t = rp.tile([CH, 1], f32)
                    nc.vector.reciprocal(r_t[0:csz, :], o_p[0:csz, D:D + 1])
                    dst = o_s[0:csz, c * D:(c + 1) * D]
                    kdiv += 1
                    if kdiv % 2 == 0:
                        nc.scalar.mul(dst, o_p[0:csz, 0:D], r_t[0:csz, :])
                    else:
                        nc.vector.tensor_scalar(
                            out=dst, in0=o_p[0:csz, 0:D],
                            scalar1=r_t[0:csz, :], scalar2=None, op0=Alu.mult,
                        )
                # fused store of the full-size chunks; tail chunk separate
                dstram = y_d[t0:t0 + nfull * CH, 0:D].rearrange(
                    "(c p) d -> p c d", p=CH,
                )
                src = o_s[:, 0:nfull * D].rearrange("p (c d) -> p c d", d=D)
                nc.sync.dma_start(dstram, src)
                if nfull != len(chunks):
                    csz = chunks[-1]
                    tt = t0 + nfull * CH
                    nc.sync.dma_start(
                        y_d[tt:tt + csz, 0:D],
                        o_s[0:csz, nfull * D:(nfull + 1) * D],
                    )

    nc.compile()
    return nc


def _split3(v):
    h = v.astype(BF16)
    r = v - h.astype(np.float32)
    m = r.astype(BF16)
    l = (r - m.astype(np.float32)).astype(BF16)
    return h, m, l


def _preprocess(inputs):
    ge = np.ascontiguousarray(np.asarray(inputs["gene_expression"], dtype=np.float32))
    pad = np.asarray(inputs["pad_mask"]) != 0
    msk = np.asarray(inputs["masked_mask"]) != 0
    w1 = np.asarray(inputs["w1"], dtype=np.float32)
    b1 = np.asarray(inputs["b1"], dtype=np.float32)
    w2 = np.asarray(inputs["w2"], dtype=np.float32)
    b2 = np.asarray(inputs["b2"], dtype=np.float32)
    emb = np.asarray(inputs["emb_table"], dtype=np.float32)
    pad_e = np.asarray(inputs["pad_emb"], dtype=np.float32)
    mask_e = np.asarray(inputs["mask_emb"], dtype=np.float32)

    pad_e = pad_e.astype(BF16).astype(np.float32)
    mask_e = mask_e.astype(BF16).astype(np.float32)

    dead = pad | msk
    pen = np.where(dead, PEN_DEAD, PEN_LIVE).astype(BF16)           # (B, N)
    q = msk.astype(np.float32)                                      # (B, N)
    p = (pad & ~msk).astype(np.float32)                             # (B, N)

    # mm1 operands: exact 3-way splits
    w1h, w1m, w1l = _split3(w1)
    b1h, b1m, b1l = _split3(b1)
    xh, xm, xl = _split3(ge)                                        # (B, N) each
    w1b9 = np.ascontiguousarray(
        np.stack([w1h, w1h, w1m, w1h, w1l, w1m, b1h, b1m, b1l], axis=0)
    )                                                               # (9, 100)
    onesN = np.ones(N, BF16)

    # mm2 operands: W = w2.T + I, hi/lo split; ones row folds pen
    w2i = (w2.T + np.eye(BIN, dtype=np.float32)).astype(np.float32)
    whi = w2i.astype(BF16)
    wlo = (w2i - whi.astype(np.float32)).astype(BF16)
    whi_aug = np.ascontiguousarray(
        np.concatenate([whi, np.ones((1, BIN), BF16)], axis=0)
    )                                                               # (101, 100)

    emb_aug = np.zeros((BIN + 2, EW), np.float32)
    emb_aug[:, D] = 1.0                                             # denominator col
    emb_aug[0:BIN, 0:D] = emb
    emb_aug[BIN, 0:D] = pad_e
    emb_aug[BIN + 1, 0:D] = mask_e                                  # col D+1 stays 0

    consts = {
        "w1b": w1b9,
        "whi": whi_aug,
        "wlo": np.ascontiguousarray(wlo),
        "emb": np.ascontiguousarray(emb_aug),
        "b2": np.ascontiguousarray(b2[:, None]),
    }
    in_maps = []
    for b in range(B):
        m = dict(consts)
        m["xb"] = np.ascontiguousarray(
            np.stack(
                [xh[b], xm[b], xh[b], xl[b], xh[b], xm[b], onesN, onesN, onesN],
                axis=0,
            )
        )
        m["pen"] = np.ascontiguousarray(pen[b][None, :])
        m["pq"] = np.ascontiguousarray(np.stack([p[b], q[b]], axis=0))
        in_maps.append(m)
    return in_maps


def _run(inputs, trace=False, trace_cores=None, **kw):
    from concourse.bass_utils import run_bass_kernel_spmd

    key = "v3"
    if key not in _prog_cache:
        _prog_cache[key] = _build_program()
    nc = _prog_cache[key]
    in_maps = _preprocess(inputs)
    res = run_bass_kernel_spmd(
        nc, in_maps, core_ids=list(range(B)),
        trace=trace, trace_cores=trace_cores, **kw,
    )
    out = np.stack([res.results[b]["y"] for b in range(B)], axis=0)
    return out, res


def kernel(**inputs):
    out, _ = _run(inputs, trace=False)
    return out


# revision 8
# speedup vs baseline: 1.5060x; 1.0486x over previous
"""Trainium2 Bass kernel for ExpressionAutoDiscretization (embedding_lookup).

Reference computation, per token t (B=8, N=19264, BIN=100, D=768):
    v1 = x_t * w1 + b1                      # (100,)
    v2 = leaky_relu(v1, 0.1)
    v3 = v2 + w2 @ v2 + b2
    w  = softmax(v3)
    e  = w @ emb_table                      # (768,)
    e  = pad_emb.bf16  if pad_mask  else e
    e  = mask_emb.bf16 if masked_mask else e   (mask wins over pad)

Kernel strategy (pure data parallel, batch row b -> core b), bins on the
SBUF partition axis, tokens on the free axis:
  * mm1 (bf16, K=9): exact 3-way bf16 splits of x, w1, b1 (hi/mid/lo) give
    v1+b1 in fp32 PSUM to ~2^-24 — one 1-cyc/row matmul instead of fp32.
  * leaky relu z=v1+b1: a=0.1z (DVE), scr=max(z,a) fp32 (DVE),
    v2h=bf16(scr) (ACT copy), v2l=bf16(scr-v2h) (DVE).
  * mm2 = three accumulating bf16 matmuls: Whi@[v2h;pen] + Whi@v2l + Wlo@v2h
    where W = (w2.T + I) hi/lo-split on host; ones row folds the per-token
    penalty row pen into v3:
       pen = -5      live tokens (softmax shift, exactness-invariant)
       pen = -30000  pad/mask tokens (zeroes their exp weights)
  * ACT Exp(v3 + b2) -> float32r E rows 0..99 of a [102,T] tile; rows
    100/101 are host indicator rows p = pad&!mask, q = mask.
  * mm3 (float32r): lhsT = E[:, chunk] (102,<=128), rhs = emb_aug (102,770):
       emb_aug = [[emb_table, 1, 0], [pad_emb.bf16, 1, 0], [mask_emb.bf16, 1, 0]]
    -> out_psum [chunk, 770]; col 768 is the softmax denominator (== 1 for
    masked tokens, making them bit-exact pad/mask embeddings).
  * DVE reciprocal of col 768, per-partition multiply (alternating ACT/DVE),
    fused 3D-AP output DMA per supertile.
"""

import numpy as np
import ml_dtypes

BF16 = ml_dtypes.bfloat16
B = 8
N = 19264          # tokens per core (= one batch row)
BIN = 100
D = 768
EW = D + 2         # emb_aug width: 768 data + denom col + even-N pad
ST = 512           # main supertile; 37 * 512 + 320 tail
CH = 128           # mm3 output chunk (partition dim)
PEN_LIVE = -5.0
PEN_DEAD = -30000.0

_prog_cache = {}


def _blocks():
    out = []
    t0 = 0
    while t0 + ST <= N:
        out.append((t0, ST, [CH] * (ST // CH)))
        t0 += ST
    rem = N - t0
    if rem:
        chunks = [CH] * (rem // CH)
        if rem % CH:
            chunks.append(rem % CH)
        out.append((t0, rem, chunks))
    return out


def _build_program():
    import concourse.bacc as bacc
    import concourse.mybir as mybir
    import concourse.tile as tile

    f32 = mybir.dt.float32
    f32r = mybir.dt.float32r
    bf16 = mybir.dt.bfloat16
    AF = mybir.ActivationFunctionType
    Alu = mybir.AluOpType

    nc = bacc.Bacc(
        "TRN2",
        target_bir_lowering=False,
        debug=False,
        enable_asserts=True,
        num_devices=B,
    )

    xb_d = nc.dram_tensor("xb", [9, N], bf16, kind="ExternalInput")
    pen_d = nc.dram_tensor("pen", [1, N], bf16, kind="ExternalInput")
    pq_d = nc.dram_tensor("pq", [2, N], f32r, kind="ExternalInput")
    w1b_d = nc.dram_tensor("w1b", [9, BIN], bf16, kind="ExternalInput")
    whi_d = nc.dram_tensor("whi", [BIN + 1, BIN], bf16, kind="ExternalInput")
    wlo_d = nc.dram_tensor("wlo", [BIN, BIN], bf16, kind="ExternalInput")
    emb_d = nc.dram_tensor("emb", [BIN + 2, EW], f32r, kind="ExternalInput")
    b2_d = nc.dram_tensor("b2", [BIN, 1], f32, kind="ExternalInput")
    y_d = nc.dram_tensor("y", [N, D], f32, kind="ExternalOutput")

    with tile.TileContext(nc) as tc:
        with (
            tc.tile_pool(name="consts", bufs=1) as consts,
            tc.tile_pool(name="xp", bufs=4) as xp,
            tc.tile_pool(name="v2hp", bufs=3) as v2hp,
            tc.tile_pool(name="v2lp", bufs=3) as v2lp,
            tc.tile_pool(name="ep", bufs=3) as ep,
            tc.tile_pool(name="ap", bufs=3) as ap_,
            tc.tile_pool(name="scrp", bufs=3) as scrp,
            tc.tile_pool(name="outs", bufs=3) as outs,
            tc.tile_pool(name="rp", bufs=8) as rp,
            tc.tile_pool(name="v1ps", bufs=1, space="PSUM") as v1ps,
            tc.tile_pool(name="v3ps", bufs=1, space="PSUM") as v3ps,
            tc.tile_pool(name="ops", bufs=3, space="PSUM") as ops,
        ):
            w1b_t = consts.tile([9, BIN], bf16)
            whi_t = consts.tile([BIN + 1, BIN], bf16)
            wlo_t = consts.tile([BIN, BIN], bf16)
            emb_t = consts.tile([BIN + 2, EW], f32r)
            b2_t = consts.tile([BIN, 1], f32)
            v2h_warm = consts.tile([BIN + 1, ST], bf16)
            nc.gpsimd.memset(v2h_warm[:], 0.0)
            nc.sync.dma_start(w1b_t[:], w1b_d[:])
            nc.sync.dma_start(whi_t[:], whi_d[:])
            nc.sync.dma_start(wlo_t[:], wlo_d[:])
            nc.sync.dma_start(emb_t[:], emb_d[:])
            nc.sync.dma_start(b2_t[:], b2_d[:])

            # dense PE warm-up burst: ~10us of back-to-back matmuls to trip
            # the HAM un-throttle (K=4/8 -> 8/8) before the real pipeline.
            wu_p = ops.tile([CH, EW], f32, tag="o_p")
            for _ in range(24):
                nc.tensor.matmul(
                    wu_p[0:BIN, 0:ST], whi_t[:], v2h_warm[:], start=True, stop=True,
                )

            kdiv = 0
            for t0, st, chunks in _blocks():
                xb_t = xp.tile([9, ST], bf16)
                nc.gpsimd.dma_start(xb_t[:, 0:st], xb_d[0:9, t0:t0 + st])

                v2h_t = v2hp.tile([BIN + 1, ST], bf16)
                nc.gpsimd.dma_start(v2h_t[BIN:BIN + 1, 0:st], pen_d[0:1, t0:t0 + st])

                e_t = ep.tile([BIN + 2, ST], f32r)
                nc.gpsimd.dma_start(e_t[BIN:BIN + 2, 0:st], pq_d[0:2, t0:t0 + st])

                # mm1: z = v1 + b1 (exact via 3-way bf16 splits) -> [100, st]
                v1_p = v1ps.tile([BIN, ST], f32)
                nc.tensor.matmul(
                    v1_p[:, 0:st], w1b_t[:], xb_t[:, 0:st], start=True, stop=True,
                )

                # leaky relu + bf16 hi/lo split of v2
                a_t = ap_.tile([BIN, ST], f32)
                nc.vector.tensor_scalar(
                    out=a_t[:, 0:st], in0=v1_p[:, 0:st], scalar1=0.1, scalar2=None,
                    op0=Alu.mult,
                )
                scr_t = scrp.tile([BIN, ST], f32)
                nc.vector.tensor_tensor(
                    out=scr_t[:, 0:st], in0=v1_p[:, 0:st], in1=a_t[:, 0:st],
                    op=Alu.max,
                )
                nc.scalar.copy(v2h_t[0:BIN, 0:st], scr_t[:, 0:st])
                v2l_t = v2lp.tile([BIN, ST], bf16)
                nc.vector.tensor_tensor(
                    out=v2l_t[:, 0:st], in0=scr_t[:, 0:st], in1=v2h_t[0:BIN, 0:st],
                    op=Alu.subtract,
                )

                # mm2: v3 = Whi@[v2h;pen] + Whi@v2l + Wlo@v2h  -> [100, st]
                v3_p = v3ps.tile([BIN, ST], f32)
                nc.tensor.matmul(
                    v3_p[:, 0:st], whi_t[:], v2h_t[:, 0:st], start=True, stop=False,
                )
                nc.tensor.matmul(
                    v3_p[:, 0:st], whi_t[0:BIN, :], v2l_t[:, 0:st],
                    start=False, stop=False,
                )
                nc.tensor.matmul(
                    v3_p[:, 0:st], wlo_t[:], v2h_t[0:BIN, 0:st],
                    start=False, stop=True,
                )

                # E = exp(v3 + b2) -> float32r
                nc.scalar.activation(
                    e_t[0:BIN, 0:st], v3_p[:, 0:st], AF.Exp, bias=b2_t[:],
                )

                # mm3 + softmax divide per chunk
                nfull = sum(1 for c in chunks if c == CH)
                o_s = outs.tile([CH, len(chunks) * D], f32)
                for c, csz in enumerate(chunks):
                    cs = c * CH
                    o_p = ops.tile([CH, EW], f32, tag="o_p")
                    nc.tensor.matmul(
                        o_p[0:csz, 0:512], e_t[:, cs:cs + csz],
                        emb_t[:, 0:512], start=True, stop=True,
                    )
                    nc.tensor.matmul(
                        o_p[0:csz, 512:EW], e_t[:, cs:cs + csz],
                        emb_t[:, 512:EW], start=True, stop=True,
                    )
                    r_t = rp.tile([CH, 1], f32)
                    nc.vector.reciprocal(r_t[0:csz, :], o_p[0:csz, D:D + 1])
                    dst = o_s[0:csz, c * D:(c + 1) * D]
                    kdiv += 1
                    if kdiv % 2 == 0:
                        nc.scalar.mul(dst, o_p[0:csz, 0:D], r_t[0:csz, :])
                    else:
                        nc.vector.tensor_scalar(
                            out=dst, in0=o_p[0:csz, 0:D],
                            scalar1=r_t[0:csz, :], scalar2=None, op0=Alu.mult,
                        )
                # fused store of the full-size chunks; tail chunk separate
                dstram = y_d[t0:t0 + nfull * CH, 0:D].rearrange(
                    "(c p) d -> p c d", p=CH,
                )
                src = o_s[:, 0:nfull * D].rearrange("p (c d) -> p c d", d=D)
                nc.sync.dma_start(dstram, src)
                if nfull != len(chunks):
                    csz = chunks[-1]
                    tt = t0 + nfull * CH
                    nc.sync.dma_start(
                        y_d[tt:tt + csz, 0:D],
                        o_s[0:csz, nfull * D:(nfull + 1) * D],
                    )

    nc.compile()
    return nc


def _split3(v):
    h = v.astype(BF16)
    r = v - h.astype(np.float32)
    m = r.astype(BF16)
    l = (r - m.astype(np.float32)).astype(BF16)
    return h, m, l


def _preprocess(inputs):
    ge = np.ascontiguousarray(np.asarray(inputs["gene_expression"], dtype=np.float32))
    pad = np.asarray(inputs["pad_mask"]) != 0
    msk = np.asarray(inputs["masked_mask"]) != 0
    w1 = np.asarray(inputs["w1"], dtype=np.float32)
    b1 = np.asarray(inputs["b1"], dtype=np.float32)
    w2 = np.asarray(inputs["w2"], dtype=np.float32)
    b2 = np.asarray(inputs["b2"], dtype=np.float32)
    emb = np.asarray(inputs["emb_table"], dtype=np.float32)
    pad_e = np.asarray(inputs["pad_emb"], dtype=np.float32)
    mask_e = np.asarray(inputs["mask_emb"], dtype=np.float32)

    pad_e = pad_e.astype(BF16).astype(np.float32)
    mask_e = mask_e.astype(BF16).astype(np.float32)

    dead = pad | msk
    pen = np.where(dead, PEN_DEAD, PEN_LIVE).astype(BF16)           # (B, N)
    q = msk.astype(np.float32)                                      # (B, N)
    p = (pad & ~msk).astype(np.float32)                             # (B, N)

    # mm1 operands: exact 3-way splits
    w1h, w1m, w1l = _split3(w1)
    b1h, b1m, b1l = _split3(b1)
    xh, xm, xl = _split3(ge)                                        # (B, N) each
    w1b9 = np.ascontiguousarray(
        np.stack([w1h, w1h, w1m, w1h, w1l, w1m, b1h, b1m, b1l], axis=0)
    )                                                               # (9, 100)
    onesN = np.ones(N, BF16)

    # mm2 operands: W = w2.T + I, hi/lo split; ones row folds pen
    w2i = (w2.T + np.eye(BIN, dtype=np.float32)).astype(np.float32)
    whi = w2i.astype(BF16)
    wlo = (w2i - whi.astype(np.float32)).astype(BF16)
    whi_aug = np.ascontiguousarray(
        np.concatenate([whi, np.ones((1, BIN), BF16)], axis=0)
    )                                                               # (101, 100)

    emb_aug = np.zeros((BIN + 2, EW), np.float32)
    emb_aug[:, D] = 1.0                                             # denominator col
    emb_aug[0:BIN, 0:D] = emb
    emb_aug[BIN, 0:D] = pad_e
    emb_aug[BIN + 1, 0:D] = mask_e                                  # col D+1 stays 0

    consts = {
        "w1b": w1b9,
        "whi": whi_aug,
        "wlo": np.ascontiguousarray(wlo),
        "emb": np.ascontiguousarray(emb_aug),
        "b2": np.ascontiguousarray(b2[:, None]),
    }
    in_maps = []
    for b in range(B):
        m = dict(consts)
        m["xb"] = np.ascontiguousarray(
            np.stack(
                [xh[b], xm[b], xh[b], xl[b], xh[b], xm[b], onesN, onesN, onesN],
                axis=0,
            )
        )
        m["pen"] = np.ascontiguousarray(pen[b][None, :])
        m["pq"] = np.ascontiguousarray(np.stack([p[b], q[b]], axis=0))
        in_maps.append(m)
    return in_maps


def _run(inputs, trace=False, trace_cores=None, **kw):
    from concourse.bass_utils import run_bass_kernel_spmd

    key = "v4"
    if key not in _prog_cache:
        _prog_cache[key] = _build_program()
    nc = _prog_cache[key]
    in_maps = _preprocess(inputs)
    res = run_bass_kernel_spmd(
        nc, in_maps, core_ids=list(range(B)),
        trace=trace, trace_cores=trace_cores, **kw,
    )
    out = np.stack([res.results[b]["y"] for b in range(B)], axis=0)
    return out, res


def kernel(**inputs):
    out, _ = _run(inputs, trace=False)
    return out
